# revision 45
# baseline (speedup 1.0000x reference)
"""Trainium2 Bass kernel for per-head causal attention (nn_Attention_52896817217709).

Sharding: 8 cores = 4 head-groups (3 heads each) x 2 batches.
Per core, per head h (S=2048, D_MODEL=768, D_HEAD=64):
  qT = W_Q[h].T @ Xq[h].T   (K=128 contraction chains; q rows 0-63 / v rows
  vT analogous               64-127 of one combined [128,S] "qv" tile)
  kT = W_K[h].T @ Xk[h].T    ([64, S] tile)
  S^T[k, q] = kT_i.T @ qT    (K=64 contraction, scores transposed)
  causal mask: identity-matmul accumulates -30000 onto the strictly-lower
  block of each diagonal tile in PSUM (start=False), so exp -> exact 0.
  P = exp(0.125 * S^T) fp16  (scalar engine only; no DVE in the path)
  z'[d',q] = sum_i vp_i.T @ P_i  with vp_i = [v_i | 1] -> row 64 = softmax sums
  attn[q, m] = (z'^T_j.T @ [W_O; b_O/H]) * (1/sums[q]), fp16 out
The attention loop runs in two q-passes (chunks 0-1, then 2-3) so only two
z' accumulators are live; scores/exp stage one k-tile ahead of z'.  All
independent projection / output-projection / v-transpose work is threaded
through a filler queue that is drained one thunk per k-tile so the PE never
idles (keeps the HAM clock-gate warm).
"""
import sys
import os
import numpy as np

for _p in ("/opt/trn_rl_repo", "/root/.axon_site/_ro/trn_rl_repo"):
    if os.path.isdir(_p) and _p not in sys.path:
        sys.path.insert(0, _p)

import concourse.bass as bass
import concourse.tile as tile
from concourse import bacc, mybir
from concourse.bass_utils import run_bass_kernel_spmd

F32 = mybir.dt.float32
FP16 = mybir.dt.float16
AF = mybir.ActivationFunctionType

B, S, H, DM, DH = 2, 2048, 12, 768, 64
HPC = 3            # heads per core
NT = S // 128      # 16 k-tiles
MT = DM // 128     # 6 m-tiles
N_CORES = 8
MASKVAL = -30000.0


def build_program():
    nc = bacc.Bacc("TRN2", target_bir_lowering=False, debug=False)

    xq = nc.dram_tensor("xq", [HPC, DM, S], FP16, kind="ExternalInput")
    xk = nc.dram_tensor("xk", [HPC, DM, S], FP16, kind="ExternalInput")
    xv = nc.dram_tensor("xv", [HPC, DM, S], FP16, kind="ExternalInput")
    wq = nc.dram_tensor("wq", [HPC, MT, 128, DH], FP16, kind="ExternalInput")
    wk = nc.dram_tensor("wk", [HPC, MT, 128, DH], FP16, kind="ExternalInput")
    wv = nc.dram_tensor("wv", [HPC, MT, 128, DH], FP16, kind="ExternalInput")
    wo = nc.dram_tensor("wo", [HPC, DH, DM], FP16, kind="ExternalInput")
    bqv = nc.dram_tensor("bqv", [HPC, 128, 1], F32, kind="ExternalInput")
    bk = nc.dram_tensor("bk", [HPC, DH, 1], F32, kind="ExternalInput")
    identh = nc.dram_tensor("identh", [128, 128], FP16, kind="ExternalInput")
    maskd = nc.dram_tensor("maskd", [128, 128], FP16, kind="ExternalInput")
    out = nc.dram_tensor("out", [HPC, S, DM], FP16, kind="ExternalOutput")

    with tile.TileContext(nc) as tc:
        with (
            tc.tile_pool(name="wpool", bufs=1) as wpool,
            tc.tile_pool(name="xt", bufs=16) as xt_pool,
            tc.tile_pool(name="qv", bufs=3) as qv_pool,
            tc.tile_pool(name="kt", bufs=3) as kt_pool,
            tc.tile_pool(name="vp", bufs=3) as vp_pool,
            tc.tile_pool(name="pp", bufs=3) as p_pool,
            tc.tile_pool(name="zt", bufs=3) as zt_pool,
            tc.tile_pool(name="rc", bufs=3) as rc_pool,
            tc.tile_pool(name="ob", bufs=3) as out_pool,
            tc.tile_pool(name="ps_s", bufs=3, space="PSUM") as ps_s,
            tc.tile_pool(name="ps_pa", bufs=3, space="PSUM") as ps_pa,
            tc.tile_pool(name="ps_z", bufs=2, space="PSUM") as ps_z,
        ):
            idh_sb = wpool.tile([128, 128], FP16, name="idh_sb")
            mask_sb = wpool.tile([128, 128], FP16, name="mask_sb")

            def load_consts():
                nc.gpsimd.dma_start(idh_sb[:], identh[:])
                nc.gpsimd.dma_start(mask_sb[:], maskd[:])

            st = [dict() for _ in range(HPC)]   # per-head live tiles
            fillq = []                          # deferred PE-work thunks
            epilogue = [False]

            def fill(n):
                for _ in range(n):
                    if fillq:
                        fillq.pop(0)()

            TENS = {"q": (xq, wq), "k": (xk, wk), "v": (xv, wv)}

            CHUNK_MAJOR = [(t, c) for c in range(4) for t in ("q", "v", "k")]

            def emit_loads(h, order=CHUNK_MAJOR):
                """X chunk loads on sync (HWDGE); weights/biases on gpsimd.
                Chunk-major order so proj_qv(h, c) unblocks earliest."""
                for t in ("q", "v", "k"):
                    st[h][f"x{t}"] = [None] * 4
                    wt = wpool.tile([128, MT, DH], FP16, name=f"w{t}{h}")
                    nc.gpsimd.dma_start(wt[:],
                                        TENS[t][1][h].rearrange("a p d -> p a d"))
                    st[h][f"w{t}"] = wt
                for t, c in order:
                    xd = TENS[t][0]
                    xc = xt_pool.tile([128, MT, 512], FP16,
                                      name=f"x{t}{h}{c}", tag="xt")
                    if h == 0 and c < 2:
                        # prologue: half-chunk DMAs complete sooner, letting
                        # the narrow first projections start earlier
                        for hf in range(2):
                            nc.sync.dma_start(
                                xc[:, :, bass.ts(hf, 256)],
                                xd[h].rearrange("(a p) s -> p a s", p=128)
                                     [:, :, 512 * c + 256 * hf:
                                      512 * c + 256 * hf + 256])
                    else:
                        nc.sync.dma_start(
                            xc[:],
                            xd[h].rearrange("(a p) s -> p a s", p=128)
                                 [:, :, bass.ts(c, 512)])
                    st[h][f"x{t}"][c] = xc
                bqv_t = wpool.tile([128, 1], F32, name=f"bqv{h}")
                nc.gpsimd.dma_start(bqv_t[:], bqv[h])
                bk_t = wpool.tile([DH, 1], F32, name=f"bk{h}")
                nc.gpsimd.dma_start(bk_t[:], bk[h])
                wot = wpool.tile([DH, DM], FP16, name=f"wo{h}")
                nc.gpsimd.dma_start(wot[:], wo[h])
                st[h]["bqv"] = bqv_t
                st[h]["bk"] = bk_t
                st[h]["wo"] = wot

            def alloc_proj(h):
                qv = qv_pool.tile([128, S], FP16, name=f"qv{h}", tag="qv")
                kT = kt_pool.tile([DH, S], FP16, name=f"kT{h}", tag="kT")
                vp2 = vp_pool.tile([128, 4, 4, DH + 1], FP16, name=f"vp{h}",
                                   tag="vp")
                nc.gpsimd.memset(vp2[:, :, :, DH:DH + 1], 1.0)
                st[h]["qv"] = qv
                st[h]["kT"] = kT
                st[h]["vp2"] = vp2

            def proj_qv(h, c):
                """q (rows 0-63) + v (rows 64-127) projection for x-chunk c."""
                qv, bqv_t = st[h]["qv"], st[h]["bqv"]
                wq_t, wv_t = st[h]["wq"], st[h]["wv"]
                xq_c, xv_c = st[h]["xq"][c], st[h]["xv"][c]
                acc = ps_pa.tile([128, 512], F32, name=f"aqv{h}{c}", tag="pa")
                for mt in range(MT):
                    nc.tensor.matmul(acc[0:DH, :], wq_t[:, mt, :],
                                     xq_c[:, mt, :],
                                     start=(mt == 0), stop=(mt == MT - 1),
                                     tile_position=(0, 0))
                    nc.tensor.matmul(acc[DH:128, :], wv_t[:, mt, :],
                                     xv_c[:, mt, :],
                                     start=(mt == 0), stop=(mt == MT - 1),
                                     tile_position=(0, DH))
                nc.vector.tensor_scalar_add(qv[:, bass.ts(c, 512)], acc[:],
                                            bqv_t[:])

            def proj_k(h, c):
                kT, bk_t, wk_t = st[h]["kT"], st[h]["bk"], st[h]["wk"]
                xk_c = st[h]["xk"][c]
                acc = ps_pa.tile([128, 512], F32, name=f"ak{h}{c}", tag="pa")
                for mt in range(MT):
                    nc.tensor.matmul(acc[0:DH, :], wk_t[:, mt, :],
                                     xk_c[:, mt, :],
                                     start=(mt == 0), stop=(mt == MT - 1),
                                     tile_position=(0, 0))
                nc.vector.tensor_scalar_add(kT[:, bass.ts(c, 512)],
                                            acc[0:DH, :], bk_t[:])

            def proj_q_solo(h, c, hf=None):
                """q-only projection (prologue: no xv dependency); hf
                restricts to a 256-wide half of the chunk."""
                qv, bqv_t, wq_t = st[h]["qv"], st[h]["bqv"], st[h]["wq"]
                xq_c = st[h]["xq"][c]
                lo, w = (0, 512) if hf is None else (256 * hf, 256)
                acc = ps_pa.tile([128, 512], F32, name=f"aq{h}{c}", tag="pa")
                for mt in range(MT):
                    nc.tensor.matmul(acc[0:DH, 0:w], wq_t[:, mt, :],
                                     xq_c[:, mt, lo:lo + w],
                                     start=(mt == 0), stop=(mt == MT - 1),
                                     tile_position=(0, 0))
                nc.vector.tensor_scalar_add(
                    qv[0:DH, 512 * c + lo:512 * c + lo + w],
                    acc[0:DH, 0:w], bqv_t[0:DH])

            def proj_v_solo(h, c, hf=None):
                qv, bqv_t, wv_t = st[h]["qv"], st[h]["bqv"], st[h]["wv"]
                xv_c = st[h]["xv"][c]
                lo, w = (0, 512) if hf is None else (256 * hf, 256)
                acc = ps_pa.tile([128, 512], F32, name=f"av{h}{c}", tag="pa")
                for mt in range(MT):
                    nc.tensor.matmul(acc[DH:128, 0:w], wv_t[:, mt, :],
                                     xv_c[:, mt, lo:lo + w],
                                     start=(mt == 0), stop=(mt == MT - 1),
                                     tile_position=(0, DH))
                nc.vector.tensor_scalar_add(
                    qv[DH:128, 512 * c + lo:512 * c + lo + w],
                    acc[DH:128, 0:w], bqv_t[DH:128])

            def vpT(h, g):
                """Transpose v k-tiles 4g..4g+3 into vp2 slot g (PE + 1 copy)."""
                qv, vp2 = st[h]["qv"], st[h]["vp2"]
                v_ps = ps_pa.tile([128, 4, DH], FP16, name=f"vps{h}{g}",
                                  tag="pa", padded_shape=[128, 4, 256])
                for t in range(4):
                    i = 4 * g + t
                    nc.tensor.transpose(v_ps[:, t, :],
                                        qv[DH:128, bass.ts(i, 128)],
                                        idh_sb[DH:128, DH:128])
                nc.vector.tensor_copy(vp2[:, g, :, 0:DH], v_ps[:])

            def outproj(h, j):
                """Output projection for q-tile j + 1/sums scale + store.
                The 512-wide evacuation goes to scalar on alternating j (the
                scalar engine has slack); the 256-wide one stays on DVE."""
                zT, rc, wot = st[h]["zT"], st[h]["rc"], st[h]["wo"]
                if j % 2 == 0:
                    st[h]["ob"] = out_pool.tile([128, 2, DM], FP16,
                                                name=f"ob{h}{j}", tag="ob")
                ob = st[h]["ob"]
                for (mo, mw) in ((0, 512), (512, 256)):
                    a_ps = ps_pa.tile([128, 512], F32, name=f"a{h}{j}{mo}",
                                      tag="pa")
                    nc.tensor.matmul(a_ps[:, 0:mw], zT[0:DH, bass.ts(j, 128)],
                                     wot[:, mo:mo + mw], start=True, stop=True)
                    use_scalar = (j % 2 == 0 if epilogue[0]
                                  else (mo == 0 and j % 2 == 0))
                    if use_scalar:
                        nc.scalar.activation(ob[:, j % 2, mo:mo + mw],
                                             a_ps[:, 0:mw], AF.Copy,
                                             scale=rc[:, j:j + 1])
                    else:
                        nc.vector.tensor_scalar_mul(ob[:, j % 2, mo:mo + mw],
                                                    a_ps[:, 0:mw],
                                                    rc[:, j:j + 1])
                if j % 2 == 1:
                    nc.gpsimd.dma_start(
                        out[h, bass.ts(j // 2, 256), :]
                           .rearrange("(a p) m -> p a m", p=128),
                        ob[:])

            def emit_B(h):
                """Two-pass causal attention for head h."""
                qv, kT, vp2 = st[h]["qv"], st[h]["kT"], st[h]["vp2"]
                zT = zt_pool.tile([DH + 1, S], FP16, name=f"zT{h}", tag="zT")
                rc = rc_pool.tile([128, NT], F32, name=f"rc{h}", tag="rc")
                st[h]["zT"] = zT
                st[h]["rc"] = rc

                for qlo_p, qhi_p in ((0, 1024), (1024, 2048)):
                    ntiles = qhi_p // 128
                    z_ps = {}
                    for c in range(qlo_p // 512, qhi_p // 512):
                        z_ps[c] = ps_z.tile([DH + 1, 512], F32,
                                            name=f"z{h}{c}", tag="z")

                    def stage(i):
                        """Scores (K=64) + mask-accumulate + exp for k-tile i."""
                        qlo = max(qlo_p, 128 * i)
                        qw = qhi_p - qlo
                        P = p_pool.tile([128, 1024], FP16, name=f"P{h}{i}",
                                        tag="P")
                        is_diag = 128 * i >= qlo_p
                        for co in range(0, qw, 512):
                            w = min(512, qw - co)
                            s_ps = ps_s.tile([128, 512], F32,
                                             name=f"s{h}{i}{co}", tag="s")
                            nc.tensor.matmul(
                                s_ps[:, 0:w], kT[:, bass.ts(i, 128)],
                                qv[0:DH, qlo + co:qlo + co + w],
                                start=True, stop=True)
                            nc.scalar.activation(P[:, co:co + w], s_ps[:, 0:w],
                                                 AF.Exp, scale=0.125)
                        if is_diag:
                            # causal mask of the diagonal block, off the PE
                            # critical path (z' consumes it one k-tile later,
                            # and only its 128-wide piece waits on this)
                            nc.vector.tensor_mul(P[:, 0:128], P[:, 0:128],
                                                 mask_sb[:])
                        return P, qlo

                    cur = stage(0)
                    pending_rc = None
                    for i in range(ntiles):
                        nxt = stage(i + 1) if i + 1 < ntiles else None
                        if pending_rc is not None:
                            pending_rc()
                            pending_rc = None
                        P, qlo = cur
                        vsl = vp2[:, i // 4, i % 4, :]
                        for c in range(qlo_p // 512, qhi_p // 512):
                            cs = max(512 * c, 128 * i)
                            w = 512 * (c + 1) - cs
                            if w <= 0:
                                continue
                            # split off the masked diagonal 128 cols so the
                            # rest of the chunk doesn't wait on the mask mul
                            parts = ([(cs, 128), (cs + 128, w - 128)]
                                     if (cs == 128 * i and 128 * i >= qlo_p
                                         and w > 128) else [(cs, w)])
                            for pn, (ps0, pw) in enumerate(parts):
                                nc.tensor.matmul(
                                    z_ps[c][:, ps0 - 512 * c:ps0 - 512 * c + pw],
                                    vsl, P[:, ps0 - qlo:ps0 - qlo + pw],
                                    start=(i == 0),
                                    stop=(i == 4 * c + 3
                                          and pn == len(parts) - 1))
                            if i == 4 * c + 3:
                                nc.vector.tensor_copy(zT[:, bass.ts(c, 512)],
                                                      z_ps[c][:])

                                def rc_thunk(c=c):
                                    rc_ps = ps_pa.tile(
                                        [128, 4, 2], FP16, name=f"rcp{h}{c}",
                                        tag="pa", padded_shape=[128, 4, 256])
                                    for a in range(4):
                                        j = 4 * c + a
                                        nc.tensor.transpose(
                                            rc_ps[:, a, 0:1],
                                            zT[DH:DH + 1, bass.ts(j, 128)],
                                            idh_sb[DH:DH + 1, DH:DH + 1])
                                    nc.vector.reciprocal(
                                        rc[:, 4 * c:4 * c + 4],
                                        rc_ps[:, :, 0])
                                    if h == HPC - 1:
                                        # last head: its output projection
                                        # self-schedules (no next B to host it)
                                        for a2 in range(4):
                                            fillq.append(
                                                lambda j=4 * c + a2:
                                                outproj(h, j))
                                # defer one k-tile so the PE isn't stalled
                                # on the zT DVE copy it depends on
                                pending_rc = rc_thunk
                        fill(1)
                        cur = nxt
                    if pending_rc is not None:
                        pending_rc()
                    fill(2)

            # ---- schedule ----
            # Prologue: load order puts B(0) pass A's dependencies first
            # (q chunks 0-1 whole, k0, v0); q/v unpaired to start sooner.
            emit_loads(0, order=[("q", 0), ("q", 1), ("k", 0), ("v", 0),
                                 ("k", 1), ("v", 1), ("q", 2), ("v", 2),
                                 ("k", 2), ("q", 3), ("v", 3), ("k", 3)])
            load_consts()
            alloc_proj(0)
            proj_q_solo(0, 0, 0)
            proj_q_solo(0, 0, 1)
            proj_q_solo(0, 1, 0)
            proj_q_solo(0, 1, 1)
            proj_k(0, 0)
            proj_v_solo(0, 0)
            vpT(0, 0)
            fillq.extend([
                lambda: proj_k(0, 1),
                lambda: proj_v_solo(0, 1),
                lambda: vpT(0, 1),
                lambda: proj_qv(0, 2),
                lambda: proj_k(0, 2),
                lambda: proj_qv(0, 3),
                lambda: proj_k(0, 3),
                lambda: vpT(0, 2),
                lambda: vpT(0, 3),
            ])

            for h in range(HPC):
                nxt = h + 1
                thunks = []
                if nxt < HPC:
                    emit_loads(nxt)
                    alloc_proj(nxt)
                    for c in range(4):
                        thunks.append(lambda n=nxt, c=c: proj_qv(n, c))
                        thunks.append(lambda n=nxt, c=c: proj_k(n, c))
                        if c % 2 == 1:
                            thunks.append(lambda n=nxt, g=c - 1: vpT(n, g))
                            thunks.append(lambda n=nxt, g=c: vpT(n, g))
                if h >= 1:
                    prev = h - 1
                    merged = []
                    for a in range(16):
                        if a < len(thunks):
                            merged.append(thunks[a])
                        merged.append(lambda p=prev, j=a: outproj(p, j))
                    thunks = merged
                fillq.extend(thunks)
                emit_B(h)
            # drain: remaining output projections of the last head; scalar
            # is idle now, so split the PSUM evacuations across both engines
            epilogue[0] = True
            fill(len(fillq) + 8)
    nc.compile()
    return nc


_CACHED = None


def _program():
    global _CACHED
    if _CACHED is None:
        _CACHED = build_program()
    return _CACHED


def _make_in_maps(inputs):
    xq_f = np.asarray(inputs["normalized_resid_pre_q"], dtype=np.float32)
    xk_f = np.asarray(inputs["normalized_resid_pre_k"], dtype=np.float32)
    xv_f = np.asarray(inputs["normalized_resid_pre_v"], dtype=np.float32)
    WQ = np.asarray(inputs["W_Q"], dtype=np.float32)
    WK = np.asarray(inputs["W_K"], dtype=np.float32)
    WV = np.asarray(inputs["W_V"], dtype=np.float32)
    WO = np.asarray(inputs["W_O"], dtype=np.float32)
    bQ = np.asarray(inputs["b_Q"], dtype=np.float32)
    bK = np.asarray(inputs["b_K"], dtype=np.float32)
    bV = np.asarray(inputs["b_V"], dtype=np.float32)
    bO = np.asarray(inputs["b_O"], dtype=np.float32)

    identh = np.eye(128, dtype=np.float16)
    maskd = (np.arange(128)[:, None] <= np.arange(128)[None, :]
             ).astype(np.float16)

    in_maps = []
    for cid in range(N_CORES):
        b = cid % 2
        hg = cid // 2
        hs = slice(HPC * hg, HPC * hg + HPC)
        bqv_h = np.concatenate(
            [bQ[hs].reshape(HPC, DH, 1), bV[hs].reshape(HPC, DH, 1)], axis=1)
        m = {
            "xq": np.ascontiguousarray(
                xq_f[b, :, hs, :].transpose(1, 2, 0)).astype(np.float16),
            "xk": np.ascontiguousarray(
                xk_f[b, :, hs, :].transpose(1, 2, 0)).astype(np.float16),
            "xv": np.ascontiguousarray(
                xv_f[b, :, hs, :].transpose(1, 2, 0)).astype(np.float16),
            "wq": np.ascontiguousarray(
                WQ[hs].reshape(HPC, MT, 128, DH)).astype(np.float16),
            "wk": np.ascontiguousarray(
                WK[hs].reshape(HPC, MT, 128, DH)).astype(np.float16),
            "wv": np.ascontiguousarray(
                WV[hs].reshape(HPC, MT, 128, DH)).astype(np.float16),
            "wo": np.ascontiguousarray(WO[hs]).astype(np.float16),
            "bqv": np.ascontiguousarray(bqv_h),
            "bk": np.ascontiguousarray(bK[hs].reshape(HPC, DH, 1)),
            "identh": identh,
            "maskd": maskd,
        }
        in_maps.append(m)
    return in_maps


def run(inputs, trace=False, **kw):
    nc = _program()
    in_maps = _make_in_maps(inputs)
    res = run_bass_kernel_spmd(nc, in_maps, core_ids=list(range(N_CORES)),
                               trace=trace, **kw)
    full = np.zeros((B, S, H, DM), np.float32)
    for cid in range(N_CORES):
        b = cid % 2
        hg = cid // 2
        o = res.results[cid]["out"]
        for j in range(HPC):
            full[b, :, HPC * hg + j, :] = o[j].astype(np.float32)
    # b_O applied on host: attn_out = (per-head proj) + b_O / n_heads
    bO = np.asarray(inputs["b_O"], dtype=np.float32)
    full += bO / H
    return full, res


def kernel(**inputs):
    full, _ = run(inputs)
    return full


# revision 46
# speedup vs baseline: 1.0899x; 1.0899x over previous
"""Trainium2 Bass kernel for per-head causal attention (nn_Attention_52896817217709).

Sharding: 8 cores = 4 head-groups (3 heads each) x 2 batches.
Per core, per head h (S=2048, D_MODEL=768, D_HEAD=64):
  qT = W_Q[h].T @ Xq[h].T   (K=128 contraction chains; q rows 0-63 / v rows
  vT analogous               64-127 of one combined [128,S] "qv" tile)
  kT = W_K[h].T @ Xk[h].T    ([64, S] tile)
  S^T[k, q] = kT_i.T @ qT    (K=64 contraction, scores transposed)
  causal mask: identity-matmul accumulates -30000 onto the strictly-lower
  block of each diagonal tile in PSUM (start=False), so exp -> exact 0.
  P = exp(0.125 * S^T) fp16  (scalar engine only; no DVE in the path)
  z'[d',q] = sum_i vp_i.T @ P_i  with vp_i = [v_i | 1] -> row 64 = softmax sums
  attn[q, m] = (z'^T_j.T @ [W_O; b_O/H]) * (1/sums[q]), fp16 out
The attention loop runs in two q-passes (chunks 0-1, then 2-3) so only two
z' accumulators are live; scores/exp stage one k-tile ahead of z'.  All
independent projection / output-projection / v-transpose work is threaded
through a filler queue that is drained one thunk per k-tile so the PE never
idles (keeps the HAM clock-gate warm).
"""
import sys
import os
import numpy as np

for _p in ("/opt/trn_rl_repo", "/root/.axon_site/_ro/trn_rl_repo"):
    if os.path.isdir(_p) and _p not in sys.path:
        sys.path.insert(0, _p)

import concourse.bass as bass
import concourse.tile as tile
from concourse import bacc, mybir
from concourse.bass_utils import run_bass_kernel_spmd

F32 = mybir.dt.float32
FP16 = mybir.dt.float16
AF = mybir.ActivationFunctionType

B, S, H, DM, DH = 2, 2048, 12, 768, 64
HPC = 3            # heads per core
NT = S // 128      # 16 k-tiles
MT = DM // 128     # 6 m-tiles
N_CORES = 8
MASKVAL = -30000.0


def build_program():
    nc = bacc.Bacc("TRN2", target_bir_lowering=False, debug=False)

    xq = nc.dram_tensor("xq", [HPC, DM, S], FP16, kind="ExternalInput")
    xk = nc.dram_tensor("xk", [HPC, DM, S], FP16, kind="ExternalInput")
    xv = nc.dram_tensor("xv", [HPC, DM, S], FP16, kind="ExternalInput")
    wq = nc.dram_tensor("wq", [HPC, MT, 128, DH], FP16, kind="ExternalInput")
    wk = nc.dram_tensor("wk", [HPC, MT, 128, DH], FP16, kind="ExternalInput")
    wv = nc.dram_tensor("wv", [HPC, MT, 128, DH], FP16, kind="ExternalInput")
    wo = nc.dram_tensor("wo", [HPC, DH, DM], FP16, kind="ExternalInput")
    bqv = nc.dram_tensor("bqv", [HPC, 128, 1], F32, kind="ExternalInput")
    bk = nc.dram_tensor("bk", [HPC, DH, 1], F32, kind="ExternalInput")
    identh = nc.dram_tensor("identh", [128, 128], FP16, kind="ExternalInput")
    maskneg = nc.dram_tensor("maskneg", [128, 128], FP16, kind="ExternalInput")
    out = nc.dram_tensor("out", [HPC, S, DM], FP16, kind="ExternalOutput")

    with tile.TileContext(nc) as tc:
        with (
            tc.tile_pool(name="wpool", bufs=1) as wpool,
            tc.tile_pool(name="xt", bufs=16) as xt_pool,
            tc.tile_pool(name="qv", bufs=3) as qv_pool,
            tc.tile_pool(name="kt", bufs=3) as kt_pool,
            tc.tile_pool(name="vp", bufs=3) as vp_pool,
            tc.tile_pool(name="pp", bufs=3) as p_pool,
            tc.tile_pool(name="zt", bufs=3) as zt_pool,
            tc.tile_pool(name="rc", bufs=3) as rc_pool,
            tc.tile_pool(name="ob", bufs=3) as out_pool,
            tc.tile_pool(name="ps_s", bufs=3, space="PSUM") as ps_s,
            tc.tile_pool(name="ps_pa", bufs=3, space="PSUM") as ps_pa,
            tc.tile_pool(name="ps_z", bufs=2, space="PSUM") as ps_z,
        ):
            idh_sb = wpool.tile([128, 128], FP16, name="idh_sb")
            mneg_sb = wpool.tile([128, 128], FP16, name="mneg_sb")

            def load_consts():
                nc.gpsimd.dma_start(idh_sb[:], identh[:])
                nc.gpsimd.dma_start(mneg_sb[:], maskneg[:])

            st = [dict() for _ in range(HPC)]   # per-head live tiles
            fillq = []                          # deferred PE-work thunks
            epilogue = [False]

            def fill(n):
                for _ in range(n):
                    if fillq:
                        fillq.pop(0)()

            TENS = {"q": (xq, wq), "k": (xk, wk), "v": (xv, wv)}

            CHUNK_MAJOR = [(t, c) for c in range(4) for t in ("q", "v", "k")]

            def emit_loads(h, order=CHUNK_MAJOR):
                """X chunk loads on sync (HWDGE); weights/biases on gpsimd.
                Chunk-major order so proj_qv(h, c) unblocks earliest."""
                for t in ("q", "v", "k"):
                    st[h][f"x{t}"] = [None] * 4
                    wt = wpool.tile([128, MT, DH], FP16, name=f"w{t}{h}")
                    nc.gpsimd.dma_start(wt[:],
                                        TENS[t][1][h].rearrange("a p d -> p a d"))
                    st[h][f"w{t}"] = wt
                for t, c in order:
                    xd = TENS[t][0]
                    xc = xt_pool.tile([128, MT, 512], FP16,
                                      name=f"x{t}{h}{c}", tag="xt")
                    if h == 0 and c < 2:
                        # prologue: half-chunk DMAs complete sooner, letting
                        # the narrow first projections start earlier
                        for hf in range(2):
                            nc.sync.dma_start(
                                xc[:, :, bass.ts(hf, 256)],
                                xd[h].rearrange("(a p) s -> p a s", p=128)
                                     [:, :, 512 * c + 256 * hf:
                                      512 * c + 256 * hf + 256])
                    else:
                        nc.sync.dma_start(
                            xc[:],
                            xd[h].rearrange("(a p) s -> p a s", p=128)
                                 [:, :, bass.ts(c, 512)])
                    st[h][f"x{t}"][c] = xc
                bqv_t = wpool.tile([128, 1], F32, name=f"bqv{h}")
                nc.gpsimd.dma_start(bqv_t[:], bqv[h])
                bk_t = wpool.tile([DH, 1], F32, name=f"bk{h}")
                nc.gpsimd.dma_start(bk_t[:], bk[h])
                wot = wpool.tile([DH, DM], FP16, name=f"wo{h}")
                nc.gpsimd.dma_start(wot[:], wo[h])
                st[h]["bqv"] = bqv_t
                st[h]["bk"] = bk_t
                st[h]["wo"] = wot

            def alloc_proj(h):
                qv = qv_pool.tile([128, S], FP16, name=f"qv{h}", tag="qv")
                kT = kt_pool.tile([DH, S], FP16, name=f"kT{h}", tag="kT")
                vp2 = vp_pool.tile([128, 4, 4, DH + 1], FP16, name=f"vp{h}",
                                   tag="vp")
                nc.gpsimd.memset(vp2[:, :, :, DH:DH + 1], 1.0)
                st[h]["qv"] = qv
                st[h]["kT"] = kT
                st[h]["vp2"] = vp2

            def proj_qv(h, c):
                """q (rows 0-63) + v (rows 64-127) projection for x-chunk c."""
                qv, bqv_t = st[h]["qv"], st[h]["bqv"]
                wq_t, wv_t = st[h]["wq"], st[h]["wv"]
                xq_c, xv_c = st[h]["xq"][c], st[h]["xv"][c]
                acc = ps_pa.tile([128, 512], F32, name=f"aqv{h}{c}", tag="pa")
                for mt in range(MT):
                    nc.tensor.matmul(acc[0:DH, :], wq_t[:, mt, :],
                                     xq_c[:, mt, :],
                                     start=(mt == 0), stop=(mt == MT - 1),
                                     tile_position=(0, 0))
                    nc.tensor.matmul(acc[DH:128, :], wv_t[:, mt, :],
                                     xv_c[:, mt, :],
                                     start=(mt == 0), stop=(mt == MT - 1),
                                     tile_position=(0, DH))
                nc.vector.tensor_scalar_add(qv[:, bass.ts(c, 512)], acc[:],
                                            bqv_t[:])

            def proj_k(h, c):
                kT, bk_t, wk_t = st[h]["kT"], st[h]["bk"], st[h]["wk"]
                xk_c = st[h]["xk"][c]
                acc = ps_pa.tile([128, 512], F32, name=f"ak{h}{c}", tag="pa")
                for mt in range(MT):
                    nc.tensor.matmul(acc[0:DH, :], wk_t[:, mt, :],
                                     xk_c[:, mt, :],
                                     start=(mt == 0), stop=(mt == MT - 1),
                                     tile_position=(0, 0))
                nc.vector.tensor_scalar_add(kT[:, bass.ts(c, 512)],
                                            acc[0:DH, :], bk_t[:])

            def proj_q_solo(h, c, hf=None):
                """q-only projection (prologue: no xv dependency); hf
                restricts to a 256-wide half of the chunk."""
                qv, bqv_t, wq_t = st[h]["qv"], st[h]["bqv"], st[h]["wq"]
                xq_c = st[h]["xq"][c]
                lo, w = (0, 512) if hf is None else (256 * hf, 256)
                acc = ps_pa.tile([128, 512], F32, name=f"aq{h}{c}", tag="pa")
                for mt in range(MT):
                    nc.tensor.matmul(acc[0:DH, 0:w], wq_t[:, mt, :],
                                     xq_c[:, mt, lo:lo + w],
                                     start=(mt == 0), stop=(mt == MT - 1),
                                     tile_position=(0, 0))
                nc.vector.tensor_scalar_add(
                    qv[0:DH, 512 * c + lo:512 * c + lo + w],
                    acc[0:DH, 0:w], bqv_t[0:DH])

            def proj_v_solo(h, c, hf=None):
                qv, bqv_t, wv_t = st[h]["qv"], st[h]["bqv"], st[h]["wv"]
                xv_c = st[h]["xv"][c]
                lo, w = (0, 512) if hf is None else (256 * hf, 256)
                acc = ps_pa.tile([128, 512], F32, name=f"av{h}{c}", tag="pa")
                for mt in range(MT):
                    nc.tensor.matmul(acc[DH:128, 0:w], wv_t[:, mt, :],
                                     xv_c[:, mt, lo:lo + w],
                                     start=(mt == 0), stop=(mt == MT - 1),
                                     tile_position=(0, DH))
                nc.vector.tensor_scalar_add(
                    qv[DH:128, 512 * c + lo:512 * c + lo + w],
                    acc[DH:128, 0:w], bqv_t[DH:128])

            def vpT(h, g):
                """Transpose v k-tiles 4g..4g+3 into vp2 slot g (PE + 1 copy)."""
                qv, vp2 = st[h]["qv"], st[h]["vp2"]
                v_ps = ps_pa.tile([128, 4, DH], FP16, name=f"vps{h}{g}",
                                  tag="pa", padded_shape=[128, 4, 256])
                for t in range(4):
                    i = 4 * g + t
                    nc.tensor.transpose(v_ps[:, t, :],
                                        qv[DH:128, bass.ts(i, 128)],
                                        idh_sb[DH:128, DH:128])
                nc.vector.tensor_copy(vp2[:, g, :, 0:DH], v_ps[:])

            def outproj(h, j):
                """Output projection for q-tile j + 1/sums scale + store.
                The 512-wide evacuation goes to scalar on alternating j (the
                scalar engine has slack); the 256-wide one stays on DVE."""
                zT, rc, wot = st[h]["zT"], st[h]["rc"], st[h]["wo"]
                if j % 2 == 0:
                    st[h]["ob"] = out_pool.tile([128, 2, DM], FP16,
                                                name=f"ob{h}{j}", tag="ob")
                ob = st[h]["ob"]
                for (mo, mw) in ((0, 512), (512, 256)):
                    a_ps = ps_pa.tile([128, 512], F32, name=f"a{h}{j}{mo}",
                                      tag="pa")
                    nc.tensor.matmul(a_ps[:, 0:mw], zT[0:DH, bass.ts(j, 128)],
                                     wot[:, mo:mo + mw], start=True, stop=True)
                    use_scalar = (j % 2 == 0 if epilogue[0]
                                  else (mo == 0 and j % 2 == 0))
                    if use_scalar:
                        nc.scalar.activation(ob[:, j % 2, mo:mo + mw],
                                             a_ps[:, 0:mw], AF.Copy,
                                             scale=rc[:, j:j + 1])
                    else:
                        nc.vector.tensor_scalar_mul(ob[:, j % 2, mo:mo + mw],
                                                    a_ps[:, 0:mw],
                                                    rc[:, j:j + 1])
                if j % 2 == 1:
                    nc.gpsimd.dma_start(
                        out[h, bass.ts(j // 2, 256), :]
                           .rearrange("(a p) m -> p a m", p=128),
                        ob[:])

            def emit_B(h):
                """Two-pass causal attention for head h."""
                qv, kT, vp2 = st[h]["qv"], st[h]["kT"], st[h]["vp2"]
                zT = zt_pool.tile([DH + 1, S], FP16, name=f"zT{h}", tag="zT")
                rc = rc_pool.tile([128, NT], F32, name=f"rc{h}", tag="rc")
                st[h]["zT"] = zT
                st[h]["rc"] = rc

                for qlo_p, qhi_p in ((0, 1024), (1024, 2048)):
                    ntiles = qhi_p // 128
                    z_ps = {}
                    for c in range(qlo_p // 512, qhi_p // 512):
                        z_ps[c] = ps_z.tile([DH + 1, 512], F32,
                                            name=f"z{h}{c}", tag="z")

                    def stage(i):
                        """Scores (K=64) + mask-accumulate + exp for k-tile i."""
                        qlo = max(qlo_p, 128 * i)
                        qw = qhi_p - qlo
                        P = p_pool.tile([128, 1024], FP16, name=f"P{h}{i}",
                                        tag="P")
                        is_diag = 128 * i >= qlo_p
                        for co in range(0, qw, 512):
                            w = min(512, qw - co)
                            s_ps = ps_s.tile([128, 512], F32,
                                             name=f"s{h}{i}{co}", tag="s")
                            nc.tensor.matmul(
                                s_ps[:, 0:w], kT[:, bass.ts(i, 128)],
                                qv[0:DH, qlo + co:qlo + co + w],
                                start=True,
                                stop=not (is_diag and co == 0))
                            if is_diag and co == 0:
                                nc.tensor.matmul(s_ps[:, 0:128], idh_sb[:],
                                                 mneg_sb[:], start=False,
                                                 stop=True)
                            nc.scalar.activation(P[:, co:co + w], s_ps[:, 0:w],
                                                 AF.Exp, scale=0.125)
                        return P, qlo

                    cur = stage(0)
                    pending_rc = None
                    for i in range(ntiles):
                        nxt = stage(i + 1) if i + 1 < ntiles else None
                        if pending_rc is not None:
                            pending_rc()
                            pending_rc = None
                        P, qlo = cur
                        vsl = vp2[:, i // 4, i % 4, :]
                        for c in range(qlo_p // 512, qhi_p // 512):
                            cs = max(512 * c, 128 * i)
                            w = 512 * (c + 1) - cs
                            if w <= 0:
                                continue
                            nc.tensor.matmul(
                                z_ps[c][:, cs - 512 * c:cs - 512 * c + w],
                                vsl, P[:, cs - qlo:cs - qlo + w],
                                start=(i == 0), stop=(i == 4 * c + 3))
                            if i == 4 * c + 3:
                                nc.vector.tensor_copy(zT[:, bass.ts(c, 512)],
                                                      z_ps[c][:])

                                def rc_thunk(c=c):
                                    rc_ps = ps_pa.tile(
                                        [128, 4, 2], FP16, name=f"rcp{h}{c}",
                                        tag="pa", padded_shape=[128, 4, 256])
                                    for a in range(4):
                                        j = 4 * c + a
                                        nc.tensor.transpose(
                                            rc_ps[:, a, 0:1],
                                            zT[DH:DH + 1, bass.ts(j, 128)],
                                            idh_sb[DH:DH + 1, DH:DH + 1])
                                    nc.vector.reciprocal(
                                        rc[:, 4 * c:4 * c + 4],
                                        rc_ps[:, :, 0])
                                    if h == HPC - 1:
                                        # last head: its output projection
                                        # self-schedules (no next B to host it)
                                        for a2 in range(4):
                                            fillq.append(
                                                lambda j=4 * c + a2:
                                                outproj(h, j))
                                # defer one k-tile so the PE isn't stalled
                                # on the zT DVE copy it depends on
                                pending_rc = rc_thunk
                        fill(1)
                        cur = nxt
                    if pending_rc is not None:
                        pending_rc()
                    fill(2)

            # ---- schedule ----
            # Prologue: load order puts B(0) pass A's dependencies first
            # (q chunks 0-1 whole, k0, v0); q/v unpaired to start sooner.
            emit_loads(0, order=[("q", 0), ("q", 1), ("k", 0), ("v", 0),
                                 ("k", 1), ("v", 1), ("q", 2), ("v", 2),
                                 ("k", 2), ("q", 3), ("v", 3), ("k", 3)])
            load_consts()
            alloc_proj(0)
            proj_q_solo(0, 0, 0)
            proj_q_solo(0, 0, 1)
            proj_q_solo(0, 1, 0)
            proj_q_solo(0, 1, 1)
            proj_k(0, 0)
            proj_v_solo(0, 0)
            vpT(0, 0)
            fillq.extend([
                lambda: proj_k(0, 1),
                lambda: proj_v_solo(0, 1),
                lambda: vpT(0, 1),
                lambda: proj_qv(0, 2),
                lambda: proj_k(0, 2),
                lambda: proj_qv(0, 3),
                lambda: proj_k(0, 3),
                lambda: vpT(0, 2),
                lambda: vpT(0, 3),
            ])

            for h in range(HPC):
                nxt = h + 1
                thunks = []
                if nxt < HPC:
                    emit_loads(nxt)
                    alloc_proj(nxt)
                    for c in range(4):
                        thunks.append(lambda n=nxt, c=c: proj_qv(n, c))
                        thunks.append(lambda n=nxt, c=c: proj_k(n, c))
                        if c % 2 == 1:
                            thunks.append(lambda n=nxt, g=c - 1: vpT(n, g))
                            thunks.append(lambda n=nxt, g=c: vpT(n, g))
                if h >= 1:
                    prev = h - 1
                    merged = []
                    for a in range(16):
                        if a < len(thunks):
                            merged.append(thunks[a])
                        merged.append(lambda p=prev, j=a: outproj(p, j))
                    thunks = merged
                fillq.extend(thunks)
                emit_B(h)
            # drain: remaining output projections of the last head; scalar
            # is idle now, so split the PSUM evacuations across both engines
            epilogue[0] = True
            fill(len(fillq) + 8)
    nc.compile()
    return nc


_CACHED = None


def _program():
    global _CACHED
    if _CACHED is None:
        _CACHED = build_program()
    return _CACHED


def _make_in_maps(inputs):
    xq_f = np.asarray(inputs["normalized_resid_pre_q"], dtype=np.float32)
    xk_f = np.asarray(inputs["normalized_resid_pre_k"], dtype=np.float32)
    xv_f = np.asarray(inputs["normalized_resid_pre_v"], dtype=np.float32)
    WQ = np.asarray(inputs["W_Q"], dtype=np.float32)
    WK = np.asarray(inputs["W_K"], dtype=np.float32)
    WV = np.asarray(inputs["W_V"], dtype=np.float32)
    WO = np.asarray(inputs["W_O"], dtype=np.float32)
    bQ = np.asarray(inputs["b_Q"], dtype=np.float32)
    bK = np.asarray(inputs["b_K"], dtype=np.float32)
    bV = np.asarray(inputs["b_V"], dtype=np.float32)
    bO = np.asarray(inputs["b_O"], dtype=np.float32)

    identh = np.eye(128, dtype=np.float16)
    maskneg = np.where(np.arange(128)[:, None] > np.arange(128)[None, :],
                       np.float16(MASKVAL), np.float16(0.0))

    in_maps = []
    for cid in range(N_CORES):
        b = cid % 2
        hg = cid // 2
        hs = slice(HPC * hg, HPC * hg + HPC)
        bqv_h = np.concatenate(
            [bQ[hs].reshape(HPC, DH, 1), bV[hs].reshape(HPC, DH, 1)], axis=1)
        m = {
            "xq": np.ascontiguousarray(
                xq_f[b, :, hs, :].transpose(1, 2, 0)).astype(np.float16),
            "xk": np.ascontiguousarray(
                xk_f[b, :, hs, :].transpose(1, 2, 0)).astype(np.float16),
            "xv": np.ascontiguousarray(
                xv_f[b, :, hs, :].transpose(1, 2, 0)).astype(np.float16),
            "wq": np.ascontiguousarray(
                WQ[hs].reshape(HPC, MT, 128, DH)).astype(np.float16),
            "wk": np.ascontiguousarray(
                WK[hs].reshape(HPC, MT, 128, DH)).astype(np.float16),
            "wv": np.ascontiguousarray(
                WV[hs].reshape(HPC, MT, 128, DH)).astype(np.float16),
            "wo": np.ascontiguousarray(WO[hs]).astype(np.float16),
            "bqv": np.ascontiguousarray(bqv_h),
            "bk": np.ascontiguousarray(bK[hs].reshape(HPC, DH, 1)),
            "identh": identh,
            "maskneg": maskneg,
        }
        in_maps.append(m)
    return in_maps


def run(inputs, trace=False, **kw):
    nc = _program()
    in_maps = _make_in_maps(inputs)
    res = run_bass_kernel_spmd(nc, in_maps, core_ids=list(range(N_CORES)),
                               trace=trace, **kw)
    full = np.zeros((B, S, H, DM), np.float32)
    for cid in range(N_CORES):
        b = cid % 2
        hg = cid // 2
        o = res.results[cid]["out"]
        for j in range(HPC):
            full[b, :, HPC * hg + j, :] = o[j].astype(np.float32)
    # b_O applied on host: attn_out = (per-head proj) + b_O / n_heads
    bO = np.asarray(inputs["b_O"], dtype=np.float32)
    full += bO / H
    return full, res


def kernel(**inputs):
    full, _ = run(inputs)
    return full
